# revision 45
# baseline (speedup 1.0000x reference)
"""Multi-head attention (B=8, S=1024, E=768, H=12) on 8 trn2 NeuronCores.

Strategy: batch-parallel — core b processes batch element b end-to-end, no
collectives.  Projection/scores matmuls in bf16; the attention-weighted sum
runs in fp8 (DoubleRow) with fp32 PSUM accumulation.

Per-core dataflow (token index s/t, feature e, head h, head-dim d):
  xT[e, s]   = PE-transpose of x (48 128x128 blocks), cast to bf16
  qT[hd, s]  = WqT^T @ xT   (bias via DVE per-partition add; 1/8 folded in Wq)
  kT[hd, s]  = WkT^T @ xT
  v[t, hdA]  = xT^T @ WvT   ([h*65+d] layout, fp8 pair-interleaved over
               t-tile pairs; col h*65+64 memset to 1 -> softmax denominator;
               bv folded into bo)
  scoresT[t, s] per head = kT_h^T @ qT_h   (K=64, bf16; heads 2j/2j+1 on
               disjoint PE row groups)
  expT = exp(scoresT - 1)  (ACT, PSUM -> fp8e4 pair-interleaved SBUF; the
               shift keeps exp in e4m3 range and cancels in the ratio)
  attn[65, s] = v_h^T @ expT  (DoubleRow fp8: K=256 per pass, 4 passes;
               row 64 = denominator)
  catT[hd, s] = attn[0:64] * recip(denom)  (DVE recip + DMA bcast + mul)
  out[s, f] = catT^T @ WoT  (bf16, k-accumulated; bo (+ Wo@bv) added at the
               DVE evacuation)
"""

import os
import numpy as np
import ml_dtypes

B, S, E, H, DH = 8, 1024, 768, 12, 64
HW = DH + 1         # per-head V width (d cols + ones col)
VW = H * HW         # 780
NT = S // 128       # 8 token tiles
NE = E // 128       # 6 feature tiles
NC_ = NT // 2       # 4 t-tile pairs (DoubleRow passes)
SHIFT = 1.0         # exp(score - SHIFT): keeps exp in fp8e4 range

_cache = {}


def _split_multiwaits(nc):
    """This toolchain's walrus encodes at most one sync-wait per instruction
    (two for EventSemaphore).  Tile's epilogue can attach more; hoist the
    extras onto same-engine NOPs placed immediately before the instruction —
    the engine sequencer executes in order, so semantics are unchanged."""
    import concourse.mybir as mybir

    for bb in nc.main_func.blocks:
        out, changed = [], False
        for ins in bb.instructions:
            si = ins.sync_info
            cap = 2 if isinstance(ins, mybir.InstEventSemaphore) else 1
            if si is not None and si.on_wait and len(si.on_wait) > cap:
                waits = list(si.on_wait)
                for w_i, w in enumerate(waits[:-cap]):
                    out.append(mybir.InstNoOp(
                        name=f"{ins.name}-wsplit{w_i}",
                        engine=ins.engine,
                        sync_info=mybir.SyncInfo(on_wait=[w], on_update=[]),
                        bass_nofuse=True,
                    ))
                ins.sync_info = mybir.SyncInfo(
                    on_wait=waits[-cap:], on_update=list(si.on_update))
                changed = True
            out.append(ins)
        if changed:
            bb.instructions = out


def _dedupe_ldweights(nc):
    """Delete an InstLdweights when the immediately-preceding PE-stream
    instructions are its identical twin followed only by plain (non-transpose)
    matmuls — the weights are still resident in the array.  Only waitless,
    updateless LDWs are removed."""
    import concourse.mybir as mybir

    ndel = 0
    for bb in nc.main_func.blocks:
        out = []
        prev_key = None          # signature of weights currently in the array
        changed = False
        for ins in bb.instructions:
            if isinstance(ins, mybir.InstLdweights):
                si = ins.sync_info
                clean = not si or (not si.on_wait and not si.on_update)
                key = (str(ins.ins[0]), str(ins.tile_position),
                       str(ins.perf_mode), str(ins.is_transpose))
                if clean and key == prev_key:
                    ndel += 1
                    changed = True
                    continue
                prev_key = key
            elif isinstance(ins, mybir.InstMatmult):
                if ins.is_transpose:
                    prev_key = None   # transpose streams data into the array
            elif ins.engine == mybir.EngineType.PE:
                prev_key = None
            out.append(ins)
        if changed:
            bb.instructions = out
    return ndel


def _build_bass(split_waits=True):
    import concourse.bass as bass
    import concourse.tile as tile
    import concourse.mybir as mybir

    from concourse.masks import make_identity

    f32 = mybir.dt.float32
    f16 = mybir.dt.float16
    bf16 = mybir.dt.bfloat16
    f8 = mybir.dt.float8e4
    EXP = mybir.ActivationFunctionType.Exp
    MULT = mybir.AluOpType.mult
    ADD = mybir.AluOpType.add
    DR = mybir.MatmulPerfMode.DoubleRow

    nc = bass.Bass(trn_type="TRN2")

    x_d = nc.dram_tensor("x", [S, E], f32, kind="ExternalInput")
    wqt_d = nc.dram_tensor("wqt", [E, E], bf16, kind="ExternalInput")
    wkt_d = nc.dram_tensor("wkt", [E, E], bf16, kind="ExternalInput")
    bq_d = nc.dram_tensor("bq", [E, 1], f32, kind="ExternalInput")
    bk_d = nc.dram_tensor("bk", [E, 1], f32, kind="ExternalInput")
    wvt_d = nc.dram_tensor("wvt", [E, VW], bf16, kind="ExternalInput")
    wot_d = nc.dram_tensor("wot", [E, E], bf16, kind="ExternalInput")
    bo_d = nc.dram_tensor("bo", [1, E], f32, kind="ExternalInput")
    out_d = nc.dram_tensor("out", [S, E], f32, kind="ExternalOutput")

    from contextlib import ExitStack

    with tile.TileContext(nc) as tc, ExitStack() as ctx:
        singles = ctx.enter_context(tc.tile_pool(name="singles", bufs=1))

        ident = singles.tile([128, 128], f32)
        make_identity(nc, ident)

        bo_bc = singles.tile([128, E], f32, tag="bobc", name="bobc")
        nc.gpsimd.dma_start(out=bo_bc, in_=bo_d[0].partition_broadcast(128))

        shift_b = singles.tile([128, 1], f32, tag="shiftb", name="shiftb")
        nc.vector.memset(shift_b, -SHIFT)

        # ---- P1: x -> xT (bf16) ----
        xt = [singles.tile([128, S], bf16, tag=f"xt{j}", name=f"xt{j}")
              for j in range(NE)]

        with tc.tile_pool(name="xload", bufs=1) as xload, \
             tc.tile_pool(name="ps_xt", bufs=4, space="PSUM") as ps_xt:
            xsb = xload.tile([128, NT * E], f32, tag="x", name="xall")
            for ib in range(2):
                x_src = bass.AP(tensor=x_d, offset=ib * 4 * 128 * E,
                                ap=[[E, 128], [128 * E, 4], [1, E]])
                nc.sync.dma_start(
                    out=xsb[:, ib * 4 * E:(ib + 1) * 4 * E], in_=x_src)
            for ib in range(2):
                for j in range(NE):
                    ps = ps_xt.tile([128, 512], f32, tag="pxt",
                                    name=f"pxt{ib}_{j}")
                    for ii in range(4):
                        i = ib * 4 + ii
                        nc.tensor.transpose(
                            ps[:, ii * 128:(ii + 1) * 128],
                            xsb[:, i * E + j * 128:i * E + (j + 1) * 128],
                            ident,
                        )
                    # split the PSUM->SBUF casts between DVE and ACT
                    if (ib * NE + j) % 2 == 0:
                        nc.vector.tensor_copy(
                            xt[j][:, ib * 512:(ib + 1) * 512], ps)
                    else:
                        nc.scalar.copy(
                            xt[j][:, ib * 512:(ib + 1) * 512], ps)

        # ---- weights / biases to SBUF ----
        class WView:
            """All k-tiles of a weight in one SBUF tile (one DMA)."""
            def __init__(self, all_tile, width):
                self.all, self.width = all_tile, width

            def __getitem__(self, k):
                return _WSlice(self, k)

        class _WSlice:
            def __init__(self, v, k):
                self.v, self.k = v, k

            def __getitem__(self, idx):
                _, cols = idx
                off = self.k * self.v.width
                return self.v.all[:, off + cols.start:off + cols.stop]

        def load_w(dram, width):
            t = singles.tile([128, NE * width], bf16, tag=f"w{dram.name}",
                             name=f"w{dram.name}")
            w_src = bass.AP(tensor=dram, offset=0,
                            ap=[[width, 128], [128 * width, NE], [1, width]])
            nc.sync.dma_start(out=t, in_=w_src)
            return WView(t, width)

        wv = load_w(wvt_d, VW)
        wq = load_w(wqt_d, E)
        wk = load_w(wkt_d, E)
        wo = load_w(wot_d, E)
        bqs, bks = [], []
        for m in range(NE):
            t = singles.tile([128, 1], f32, tag=f"bq{m}", name=f"bq{m}")
            nc.sync.dma_start(out=t, in_=bq_d[m * 128:(m + 1) * 128, :])
            bqs.append(t)
            t = singles.tile([128, 1], f32, tag=f"bk{m}", name=f"bk{m}")
            nc.sync.dma_start(out=t, in_=bk_d[m * 128:(m + 1) * 128, :])
            bks.append(t)

        # ---- P2a: V projection -> fp8 paired vt ----
        # DoubleRow wants the pair dim as a 16-aligned block stride: slot s of
        # vt_p[c] (cols [s*VW8, s*VW8+780)) holds v for t-tile 2c+s.
        VW8 = (VW + 15) // 16 * 16   # 784
        vt_p = [singles.tile([128, 2 * VW8], f8, tag=f"vt{c}", name=f"vt{c}")
                for c in range(NC_)]

        # ---- P2a: V projection -> fp8 paired vt ----
        with tc.tile_pool(name="ps_v", bufs=2, space="PSUM") as ps_v:
            for i in range(NT - 2):
                ps = ps_v.tile([128, VW], f32, tag="pv", name=f"pv{i}")
                for k in range(NE):
                    for off, sz in ((0, 512), (512, VW - 512)):
                        nc.tensor.matmul(
                            ps[:, off:off + sz],
                            lhsT=xt[k][:, i * 128:(i + 1) * 128],
                            rhs=wv[k][:, off:off + sz],
                            start=(k == 0), stop=(k == NE - 1),
                        )
                slot = (i % 2) * VW8
                nc.vector.tensor_copy(
                    vt_p[i // 2][:, slot:slot + VW], ps)
            for c in range(NC_ - 1):
                ones_ap = vt_p[c].rearrange(
                    "p (two hw) -> p two hw", two=2
                )[:, :, 0:VW].rearrange(
                    "p two (h w) -> p two h w", h=H)[:, :, :, DH]
                nc.gpsimd.memset(ones_ap, 1.0)

        # ---- P2b/P3 interleaved per head-pair ----
        qt = [singles.tile([128, S], bf16, tag=f"qt{j}", name=f"qt{j}")
              for j in range(NE)]
        kt = [singles.tile([128, S], bf16, tag=f"kt{j}", name=f"kt{j}")
              for j in range(NE)]
        catt = [singles.tile([128, S], bf16, tag=f"ct{j}", name=f"ct{j}")
                for j in range(NE)]

        with tc.tile_pool(name="exp", bufs=20) as expp, \
             tc.tile_pool(name="norm", bufs=4) as normp, \
             tc.tile_pool(name="ps_proj", bufs=2, space="PSUM") as ps_proj, \
             tc.tile_pool(name="ps_sc", bufs=2, space="PSUM") as ps_sc, \
             tc.tile_pool(name="ps_at", bufs=2, space="PSUM") as ps_at, \
             tc.tile_pool(name="dscr", bufs=16, space="DRAM") as dscr:
            def emit_qk(hp):
                for dst, w, b in ((kt, wk, bks), (qt, wq, bqs)):
                    for sc in range(2):
                        ps = ps_proj.tile([128, 512], f32, tag="pp",
                                          name=f"pp{hp}_{dst[0].name}{sc}")
                        for k in range(NE):
                            nc.tensor.matmul(
                                ps,
                                lhsT=w[k][:, hp * 128:(hp + 1) * 128],
                                rhs=xt[k][:, sc * 512:(sc + 1) * 512],
                                start=(k == 0), stop=(k == NE - 1),
                            )
                        # ACT Identity+bias: offloads the congested DVE and
                        # fits in the EXP stream's gaps
                        nc.scalar.add(
                            dst[hp][:, sc * 512:(sc + 1) * 512], ps, b[hp])

            emit_qk(0)
            for hp in range(H // 2):
                # exps[half][c] = fp8 paired tile [128, (2 slots x 1024 s)]
                exps = [[None] * NC_ for _ in range(2)]
                for t in range(NT):
                    for half in range(2):
                        lo, hi = half * 64, half * 64 + 64
                        ps = ps_sc.tile([128, 1024], f32, tag="sc",
                                        name=f"sc{hp}_{t}_{half}")
                        for sc in range(2):
                            nc.tensor.matmul(
                                ps[:, sc * 512:(sc + 1) * 512],
                                lhsT=kt[hp][lo:hi, t * 128:(t + 1) * 128],
                                rhs=qt[hp][lo:hi, sc * 512:(sc + 1) * 512],
                                start=True, stop=True,
                                tile_position=(lo, 0),
                            )
                        c, slot = t // 2, t % 2
                        if exps[half][c] is None:
                            exps[half][c] = expp.tile(
                                [128, 2048], f8, tag="e",
                                name=f"e{hp}_{half}_{c}")
                        ex_ap = exps[half][c][:, slot * 1024:(slot + 1) * 1024]
                        nc.scalar.activation(ex_ap, ps, EXP, bias=shift_b)
                if hp + 1 < H // 2:
                    emit_qk(hp + 1)
                if hp == 0:
                    # last two V t-tiles fill hp0's slack (it has no
                    # previous head-pair attention to run)
                    for i in (NT - 2, NT - 1):
                        slot = (i % 2) * VW8
                        for off, sz in ((0, 512), (512, VW - 512)):
                            pv = ps_proj.tile([128, 512], f32, tag="pp",
                                              name=f"pv{i}_{off}")
                            for k in range(NE):
                                nc.tensor.matmul(
                                    pv[:, 0:sz],
                                    lhsT=xt[k][:, i * 128:(i + 1) * 128],
                                    rhs=wv[k][:, off:off + sz],
                                    start=(k == 0), stop=(k == NE - 1),
                                )
                            nc.vector.tensor_copy(
                                vt_p[i // 2][:, slot + off:slot + off + sz],
                                pv[:, 0:sz])
                    ones_ap = vt_p[NC_ - 1].rearrange(
                        "p (two hw) -> p two hw", two=2
                    )[:, :, 0:VW].rearrange(
                        "p two (h w) -> p two h w", h=H)[:, :, :, DH]
                    nc.gpsimd.memset(ones_ap, 1.0)
                for half in range(2):
                    head = hp * 2 + half
                    for sc in range(2):
                        # [65, 512] = one PSUM bank; one DR MM per pass
                        # (start=True clears the whole bank's has_written)
                        pa = ps_at.tile([HW, 512], f32, tag="at",
                                        name=f"at{head}_{sc}")
                        for c in range(NC_):
                            vt_h = vt_p[c].rearrange(
                                "p (two w) -> p two w", two=2
                            )[:, :, head * HW:(head + 1) * HW]
                            rhs = exps[half][c].rearrange(
                                "p (two s) -> p two s", two=2
                            )[:, :, sc * 512:(sc + 1) * 512]
                            nc.tensor.matmul(
                                pa, lhsT=vt_h, rhs=rhs,
                                start=(c == 0), stop=(c == NC_ - 1),
                                perf_mode=DR,
                            )
                        asb = normp.tile([HW, 512], f32, tag="asb",
                                         name=f"asb{head}_{sc}")
                        nc.vector.tensor_copy(asb, pa)
                        # DVE reciprocal costs ~6ns/elem/lane: spread the 512
                        # denominators across 128 partitions via DRAM bounce
                        dn = dscr.tile([1, 512], f32, tag="dn",
                                       name=f"dn{head}_{sc}")
                        nc.sync.dma_start(out=dn, in_=asb[64:65, :])
                        d4 = normp.tile([128, 4], f32, tag="d4",
                                        name=f"d4{head}_{sc}")
                        dn_r = bass.AP(tensor=dn.tensor, offset=dn.offset,
                                       ap=[[4, 128], [1, 4]])
                        nc.gpsimd.dma_start(out=d4, in_=dn_r)
                        r4 = normp.tile([128, 4], f32, tag="r4",
                                        name=f"r4{head}_{sc}")
                        nc.vector.reciprocal(r4, d4)
                        dn2 = dscr.tile([1, 512], f32, tag="dn2",
                                        name=f"dn2{head}_{sc}")
                        dn2_w = bass.AP(tensor=dn2.tensor, offset=dn2.offset,
                                        ap=[[4, 128], [1, 4]])
                        nc.gpsimd.dma_start(out=dn2_w, in_=r4)
                        rcb = normp.tile([64, 512], f32, tag="rcb",
                                         name=f"rcb{head}_{sc}")
                        nc.gpsimd.dma_start(
                            out=rcb, in_=dn2[0].partition_broadcast(64))
                        muleng = nc.vector if hp == H // 2 - 1 else nc.gpsimd
                        muleng.tensor_mul(
                            catt[hp][half * 64:(half + 1) * 64,
                                     sc * 512:(sc + 1) * 512],
                            asb[0:64, :], rcb)

        # ---- P4: output projection ----
        with tc.tile_pool(name="osb2", bufs=3) as osb2, \
             tc.tile_pool(name="ps_o", bufs=4, space="PSUM") as ps_o:
            for m in range(NT):
                ps = ps_o.tile([128, E], f32, tag="po", name=f"po{m}")
                for k in range(NE):
                    for off, sz in ((0, 512), (512, E - 512)):
                        nc.tensor.matmul(
                            ps[:, off:off + sz],
                            lhsT=catt[k][:, m * 128:(m + 1) * 128],
                            rhs=wo[k][:, off:off + sz],
                            start=(k == 0), stop=(k == NE - 1),
                        )
                ot = osb2.tile([128, E], f32, tag="o", name=f"ot{m}")
                nc.vector.scalar_tensor_tensor(ot, ps, 1.0, bo_bc, MULT, ADD)
                nc.sync.dma_start(out=out_d[m * 128:(m + 1) * 128, :], in_=ot)

    _dedupe_ldweights(nc)
    if split_waits:
        _split_multiwaits(nc)
    return nc


def _prep_weights(Wq, bq, Wk, bk, Wv, bv, Wo, bo):
    bf16 = ml_dtypes.bfloat16
    scale = 1.0 / np.sqrt(np.float32(DH))

    wqt = (np.asarray(Wq, np.float32).reshape(H * DH, E) * scale).T.astype(bf16)
    wkt = np.asarray(Wk, np.float32).reshape(H * DH, E).T.astype(bf16)
    bqv = (np.asarray(bq, np.float32).reshape(E, 1) * scale).astype(np.float32)
    bkv = np.asarray(bk, np.float32).reshape(E, 1).astype(np.float32)

    wvt = np.zeros((E, VW), np.float32)
    Wv = np.asarray(Wv, np.float32)
    bv = np.asarray(bv, np.float32)
    for h in range(H):
        wvt[0:E, h * HW:h * HW + DH] = Wv[h].T
    wvt = wvt.astype(bf16)

    Wo = np.asarray(Wo, np.float32)
    bo = np.asarray(bo, np.float32)
    # fold the V bias through the output projection: softmax weights sum to
    # one, so attn(v + bv) = attn(v) + bv and out += Wo @ concat(bv)
    bv_cat = bv.reshape(E)
    bo_new = (bo + Wo @ bv_cat).reshape(1, E).astype(np.float32)
    wot = Wo.T.astype(bf16)
    return wqt, wkt, bqv, bkv, wvt, wot, bo_new


def _install_ntff_shim():
    """Provide antenv.axon_hooks (absent in this image) so trace=True can
    drive NRT profiling through libaxon_pjrt.so.  Dev-only; harmless no-op
    when anything is missing."""
    import sys, types
    try:
        import antenv.axon_hooks  # noqa
        return
    except ImportError:
        pass
    try:
        import antenv
        mod = types.ModuleType("antenv.axon_hooks")
        _state = {}
        mod.set_axon_ntff_profile_hook = lambda h: _state.update(h=h)
        mod.get_axon_ntff_profile_hook = lambda: _state.get("h")
        sys.modules["antenv.axon_hooks"] = mod
        antenv.axon_hooks = mod
        from trn_agent_boot.trn_boot import _ntff_profile_via_ctypes
        hook = _ntff_profile_via_ctypes("/opt/axon/libaxon_pjrt.so")
        if hook is not None:
            mod.set_axon_ntff_profile_hook(hook)
    except Exception as e:  # pragma: no cover
        print(f"ntff shim failed: {e}")


def kernel(x, Wq, bq, Wk, bk, Wv, bv, Wo, bo):
    from concourse.bass_utils import run_bass_kernel_spmd

    if "nc" not in _cache:
        _cache["nc"] = _build_bass()
    nc = _cache["nc"]

    wqt, wkt, bqv, bkv, wvt, wot, bo_new = _prep_weights(
        Wq, bq, Wk, bk, Wv, bv, Wo, bo)
    x = np.asarray(x, np.float32)
    in_maps = [
        {"x": np.ascontiguousarray(x[b]),
         "wqt": wqt, "wkt": wkt, "bq": bqv, "bk": bkv,
         "wvt": wvt, "wot": wot, "bo": bo_new}
        for b in range(B)
    ]
    trace = bool(int(os.environ.get("MHA_TRACE", "0")))
    if trace:
        _install_ntff_shim()
    res = run_bass_kernel_spmd(nc, in_maps, list(range(B)), trace=trace)
    _cache["last_results"] = res
    return np.stack([res.results[b]["out"] for b in range(B)]).astype(np.float32)


# revision 46
# speedup vs baseline: 1.1210x; 1.1210x over previous
"""Multi-head attention (B=8, S=1024, E=768, H=12) on 8 trn2 NeuronCores.

Strategy: batch-parallel — core b processes batch element b end-to-end, no
collectives.  Projection/scores matmuls in bf16; the attention-weighted sum
runs in fp8 (DoubleRow) with fp32 PSUM accumulation.

Per-core dataflow (token index s/t, feature e, head h, head-dim d):
  xT[e, s]   = PE-transpose of x (48 128x128 blocks), cast to bf16
  qT[hd, s]  = WqT^T @ xT   (bias via DVE per-partition add; 1/8 folded in Wq)
  kT[hd, s]  = WkT^T @ xT
  v[t, hdA]  = xT^T @ WvT   ([h*65+d] layout, fp8 pair-interleaved over
               t-tile pairs; col h*65+64 memset to 1 -> softmax denominator;
               bv folded into bo)
  scoresT[t, s] per head = kT_h^T @ qT_h   (K=64, bf16; heads 2j/2j+1 on
               disjoint PE row groups)
  expT = exp(scoresT - 1)  (ACT, PSUM -> fp8e4 pair-interleaved SBUF; the
               shift keeps exp in e4m3 range and cancels in the ratio)
  attn[65, s] = v_h^T @ expT  (DoubleRow fp8: K=256 per pass, 4 passes;
               row 64 = denominator)
  catT[hd, s] = attn[0:64] * recip(denom)  (DVE recip + DMA bcast + mul)
  out[s, f] = catT^T @ WoT  (bf16, k-accumulated; bo (+ Wo@bv) added at the
               DVE evacuation)
"""

import os
import numpy as np
import ml_dtypes

B, S, E, H, DH = 8, 1024, 768, 12, 64
HW = DH + 1         # per-head V width (d cols + ones col)
VW = H * HW         # 780
NT = S // 128       # 8 token tiles
NE = E // 128       # 6 feature tiles
NC_ = NT // 2       # 4 t-tile pairs (DoubleRow passes)
SHIFT = 1.0         # exp(score - SHIFT): keeps exp in fp8e4 range

_cache = {}


def _split_multiwaits(nc):
    """This toolchain's walrus encodes at most one sync-wait per instruction
    (two for EventSemaphore).  Tile's epilogue can attach more; hoist the
    extras onto same-engine NOPs placed immediately before the instruction —
    the engine sequencer executes in order, so semantics are unchanged."""
    import concourse.mybir as mybir

    for bb in nc.main_func.blocks:
        out, changed = [], False
        for ins in bb.instructions:
            si = ins.sync_info
            cap = 2 if isinstance(ins, mybir.InstEventSemaphore) else 1
            if si is not None and si.on_wait and len(si.on_wait) > cap:
                waits = list(si.on_wait)
                for w_i, w in enumerate(waits[:-cap]):
                    out.append(mybir.InstNoOp(
                        name=f"{ins.name}-wsplit{w_i}",
                        engine=ins.engine,
                        sync_info=mybir.SyncInfo(on_wait=[w], on_update=[]),
                        bass_nofuse=True,
                    ))
                ins.sync_info = mybir.SyncInfo(
                    on_wait=waits[-cap:], on_update=list(si.on_update))
                changed = True
            out.append(ins)
        if changed:
            bb.instructions = out


def _dedupe_ldweights(nc):
    """Delete an InstLdweights when the immediately-preceding PE-stream
    instructions are its identical twin followed only by plain (non-transpose)
    matmuls — the weights are still resident in the array.  Only waitless,
    updateless LDWs are removed."""
    import concourse.mybir as mybir

    ndel = 0
    for bb in nc.main_func.blocks:
        out = []
        prev_key = None          # signature of weights currently in the array
        changed = False
        for ins in bb.instructions:
            if isinstance(ins, mybir.InstLdweights):
                si = ins.sync_info
                clean = not si or (not si.on_wait and not si.on_update)
                key = (str(ins.ins[0]), str(ins.tile_position),
                       str(ins.perf_mode), str(ins.is_transpose))
                if clean and key == prev_key:
                    ndel += 1
                    changed = True
                    continue
                prev_key = key
            elif isinstance(ins, mybir.InstMatmult):
                if ins.is_transpose:
                    prev_key = None   # transpose streams data into the array
            elif ins.engine == mybir.EngineType.PE:
                prev_key = None
            out.append(ins)
        if changed:
            bb.instructions = out
    return ndel


def _build_bass(split_waits=True):
    import concourse.bass as bass
    import concourse.tile as tile
    import concourse.mybir as mybir

    from concourse.masks import make_identity

    f32 = mybir.dt.float32
    f16 = mybir.dt.float16
    bf16 = mybir.dt.bfloat16
    f8 = mybir.dt.float8e4
    EXP = mybir.ActivationFunctionType.Exp
    MULT = mybir.AluOpType.mult
    ADD = mybir.AluOpType.add
    DR = mybir.MatmulPerfMode.DoubleRow

    nc = bass.Bass(trn_type="TRN2")

    x_d = nc.dram_tensor("x", [S, E], f32, kind="ExternalInput")
    wqt_d = nc.dram_tensor("wqt", [E, E], bf16, kind="ExternalInput")
    wkt_d = nc.dram_tensor("wkt", [E, E], bf16, kind="ExternalInput")
    bq_d = nc.dram_tensor("bq", [E, 1], f32, kind="ExternalInput")
    bk_d = nc.dram_tensor("bk", [E, 1], f32, kind="ExternalInput")
    wvt_d = nc.dram_tensor("wvt", [E, VW], bf16, kind="ExternalInput")
    wot_d = nc.dram_tensor("wot", [E, E], bf16, kind="ExternalInput")
    bo_d = nc.dram_tensor("bo", [1, E], f32, kind="ExternalInput")
    out_d = nc.dram_tensor("out", [S, E], f32, kind="ExternalOutput")

    from contextlib import ExitStack

    with tile.TileContext(nc) as tc, ExitStack() as ctx:
        singles = ctx.enter_context(tc.tile_pool(name="singles", bufs=1))

        ident = singles.tile([128, 128], f32)
        make_identity(nc, ident)

        bo_bc = singles.tile([128, E], f32, tag="bobc", name="bobc")
        nc.gpsimd.dma_start(out=bo_bc, in_=bo_d[0].partition_broadcast(128))

        shift_b = singles.tile([128, 1], f32, tag="shiftb", name="shiftb")
        nc.vector.memset(shift_b, -SHIFT)

        # ---- P1: x -> xT (bf16) ----
        xt = [singles.tile([128, S], bf16, tag=f"xt{j}", name=f"xt{j}")
              for j in range(NE)]

        with tc.tile_pool(name="xload", bufs=1) as xload, \
             tc.tile_pool(name="ps_xt", bufs=4, space="PSUM") as ps_xt:
            xsb = xload.tile([128, NT * E], f32, tag="x", name="xall")
            for ib in range(2):
                x_src = bass.AP(tensor=x_d, offset=ib * 4 * 128 * E,
                                ap=[[E, 128], [128 * E, 4], [1, E]])
                nc.sync.dma_start(
                    out=xsb[:, ib * 4 * E:(ib + 1) * 4 * E], in_=x_src)
            for ib in range(2):
                for j in range(NE):
                    ps = ps_xt.tile([128, 512], f32, tag="pxt",
                                    name=f"pxt{ib}_{j}")
                    for ii in range(4):
                        i = ib * 4 + ii
                        nc.tensor.transpose(
                            ps[:, ii * 128:(ii + 1) * 128],
                            xsb[:, i * E + j * 128:i * E + (j + 1) * 128],
                            ident,
                        )
                    # split the PSUM->SBUF casts between DVE and ACT
                    if (ib * NE + j) % 2 == 0:
                        nc.vector.tensor_copy(
                            xt[j][:, ib * 512:(ib + 1) * 512], ps)
                    else:
                        nc.scalar.copy(
                            xt[j][:, ib * 512:(ib + 1) * 512], ps)

        # ---- weights / biases to SBUF ----
        class WView:
            """All k-tiles of a weight in one SBUF tile (one DMA)."""
            def __init__(self, all_tile, width):
                self.all, self.width = all_tile, width

            def __getitem__(self, k):
                return _WSlice(self, k)

        class _WSlice:
            def __init__(self, v, k):
                self.v, self.k = v, k

            def __getitem__(self, idx):
                _, cols = idx
                off = self.k * self.v.width
                return self.v.all[:, off + cols.start:off + cols.stop]

        def load_w(dram, width):
            t = singles.tile([128, NE * width], bf16, tag=f"w{dram.name}",
                             name=f"w{dram.name}")
            w_src = bass.AP(tensor=dram, offset=0,
                            ap=[[width, 128], [128 * width, NE], [1, width]])
            nc.sync.dma_start(out=t, in_=w_src)
            return WView(t, width)

        wv = load_w(wvt_d, VW)
        wq = load_w(wqt_d, E)
        wk = load_w(wkt_d, E)
        wo = load_w(wot_d, E)
        bqs, bks = [], []
        for m in range(NE):
            t = singles.tile([128, 1], f32, tag=f"bq{m}", name=f"bq{m}")
            nc.sync.dma_start(out=t, in_=bq_d[m * 128:(m + 1) * 128, :])
            bqs.append(t)
            t = singles.tile([128, 1], f32, tag=f"bk{m}", name=f"bk{m}")
            nc.sync.dma_start(out=t, in_=bk_d[m * 128:(m + 1) * 128, :])
            bks.append(t)

        # ---- P2a: V projection -> fp8 paired vt ----
        # DoubleRow wants the pair dim as a 16-aligned block stride: slot s of
        # vt_p[c] (cols [s*VW8, s*VW8+780)) holds v for t-tile 2c+s.
        VW8 = (VW + 15) // 16 * 16   # 784
        vt_p = [singles.tile([128, 2 * VW8], f8, tag=f"vt{c}", name=f"vt{c}")
                for c in range(NC_)]

        # ---- P2a: V projection -> fp8 paired vt ----
        with tc.tile_pool(name="ps_v", bufs=2, space="PSUM") as ps_v:
            for i in range(NT - 2):
                ps = ps_v.tile([128, VW], f32, tag="pv", name=f"pv{i}")
                for k in range(NE):
                    for off, sz in ((0, 512), (512, VW - 512)):
                        nc.tensor.matmul(
                            ps[:, off:off + sz],
                            lhsT=xt[k][:, i * 128:(i + 1) * 128],
                            rhs=wv[k][:, off:off + sz],
                            start=(k == 0), stop=(k == NE - 1),
                        )
                slot = (i % 2) * VW8
                nc.vector.tensor_copy(
                    vt_p[i // 2][:, slot:slot + VW], ps)
            for c in range(NC_ - 1):
                ones_ap = vt_p[c].rearrange(
                    "p (two hw) -> p two hw", two=2
                )[:, :, 0:VW].rearrange(
                    "p two (h w) -> p two h w", h=H)[:, :, :, DH]
                nc.gpsimd.memset(ones_ap, 1.0)

        # ---- P2b/P3 interleaved per head-pair ----
        qt = [singles.tile([128, S], bf16, tag=f"qt{j}", name=f"qt{j}")
              for j in range(NE)]
        kt = [singles.tile([128, S], bf16, tag=f"kt{j}", name=f"kt{j}")
              for j in range(NE)]
        catt = [singles.tile([128, S], bf16, tag=f"ct{j}", name=f"ct{j}")
                for j in range(NE)]

        with tc.tile_pool(name="exp", bufs=20) as expp, \
             tc.tile_pool(name="norm", bufs=4) as normp, \
             tc.tile_pool(name="ps_proj", bufs=2, space="PSUM") as ps_proj, \
             tc.tile_pool(name="ps_sc", bufs=2, space="PSUM") as ps_sc, \
             tc.tile_pool(name="ps_at", bufs=2, space="PSUM") as ps_at, \
             tc.tile_pool(name="dscr", bufs=16, space="DRAM") as dscr:
            def emit_qk(hp):
                for dst, w, b in ((kt, wk, bks), (qt, wq, bqs)):
                    for sc in range(2):
                        ps = ps_proj.tile([128, 512], f32, tag="pp",
                                          name=f"pp{hp}_{dst[0].name}{sc}")
                        for k in range(NE):
                            nc.tensor.matmul(
                                ps,
                                lhsT=w[k][:, hp * 128:(hp + 1) * 128],
                                rhs=xt[k][:, sc * 512:(sc + 1) * 512],
                                start=(k == 0), stop=(k == NE - 1),
                            )
                        nc.vector.tensor_scalar_add(
                            dst[hp][:, sc * 512:(sc + 1) * 512], ps, b[hp])

            emit_qk(0)
            for hp in range(H // 2):
                # exps[half][c] = fp8 paired tile [128, (2 slots x 1024 s)]
                exps = [[None] * NC_ for _ in range(2)]
                for t in range(NT):
                    for half in range(2):
                        lo, hi = half * 64, half * 64 + 64
                        ps = ps_sc.tile([128, 1024], f32, tag="sc",
                                        name=f"sc{hp}_{t}_{half}")
                        for sc in range(2):
                            nc.tensor.matmul(
                                ps[:, sc * 512:(sc + 1) * 512],
                                lhsT=kt[hp][lo:hi, t * 128:(t + 1) * 128],
                                rhs=qt[hp][lo:hi, sc * 512:(sc + 1) * 512],
                                start=True, stop=True,
                                tile_position=(lo, 0),
                            )
                        c, slot = t // 2, t % 2
                        if exps[half][c] is None:
                            exps[half][c] = expp.tile(
                                [128, 2048], f8, tag="e",
                                name=f"e{hp}_{half}_{c}")
                        ex_ap = exps[half][c][:, slot * 1024:(slot + 1) * 1024]
                        nc.scalar.activation(ex_ap, ps, EXP, bias=shift_b)
                if hp + 1 < H // 2:
                    emit_qk(hp + 1)
                if hp == 0:
                    # last two V t-tiles fill hp0's slack (it has no
                    # previous head-pair attention to run)
                    for i in (NT - 2, NT - 1):
                        slot = (i % 2) * VW8
                        for off, sz in ((0, 512), (512, VW - 512)):
                            pv = ps_proj.tile([128, 512], f32, tag="pp",
                                              name=f"pv{i}_{off}")
                            for k in range(NE):
                                nc.tensor.matmul(
                                    pv[:, 0:sz],
                                    lhsT=xt[k][:, i * 128:(i + 1) * 128],
                                    rhs=wv[k][:, off:off + sz],
                                    start=(k == 0), stop=(k == NE - 1),
                                )
                            nc.vector.tensor_copy(
                                vt_p[i // 2][:, slot + off:slot + off + sz],
                                pv[:, 0:sz])
                    ones_ap = vt_p[NC_ - 1].rearrange(
                        "p (two hw) -> p two hw", two=2
                    )[:, :, 0:VW].rearrange(
                        "p two (h w) -> p two h w", h=H)[:, :, :, DH]
                    nc.gpsimd.memset(ones_ap, 1.0)
                for half in range(2):
                    head = hp * 2 + half
                    for sc in range(2):
                        # [65, 512] = one PSUM bank; one DR MM per pass
                        # (start=True clears the whole bank's has_written)
                        pa = ps_at.tile([HW, 512], f32, tag="at",
                                        name=f"at{head}_{sc}")
                        for c in range(NC_):
                            vt_h = vt_p[c].rearrange(
                                "p (two w) -> p two w", two=2
                            )[:, :, head * HW:(head + 1) * HW]
                            rhs = exps[half][c].rearrange(
                                "p (two s) -> p two s", two=2
                            )[:, :, sc * 512:(sc + 1) * 512]
                            nc.tensor.matmul(
                                pa, lhsT=vt_h, rhs=rhs,
                                start=(c == 0), stop=(c == NC_ - 1),
                                perf_mode=DR,
                            )
                        asb = normp.tile([HW, 512], f32, tag="asb",
                                         name=f"asb{head}_{sc}")
                        nc.vector.tensor_copy(asb, pa)
                        # DVE reciprocal costs ~6ns/elem/lane: spread the 512
                        # denominators across 128 partitions via DRAM bounce
                        dn = dscr.tile([1, 512], f32, tag="dn",
                                       name=f"dn{head}_{sc}")
                        nc.sync.dma_start(out=dn, in_=asb[64:65, :])
                        d4 = normp.tile([128, 4], f32, tag="d4",
                                        name=f"d4{head}_{sc}")
                        dn_r = bass.AP(tensor=dn.tensor, offset=dn.offset,
                                       ap=[[4, 128], [1, 4]])
                        nc.gpsimd.dma_start(out=d4, in_=dn_r)
                        r4 = normp.tile([128, 4], f32, tag="r4",
                                        name=f"r4{head}_{sc}")
                        nc.vector.reciprocal(r4, d4)
                        dn2 = dscr.tile([1, 512], f32, tag="dn2",
                                        name=f"dn2{head}_{sc}")
                        dn2_w = bass.AP(tensor=dn2.tensor, offset=dn2.offset,
                                        ap=[[4, 128], [1, 4]])
                        nc.gpsimd.dma_start(out=dn2_w, in_=r4)
                        rcb = normp.tile([64, 512], f32, tag="rcb",
                                         name=f"rcb{head}_{sc}")
                        nc.gpsimd.dma_start(
                            out=rcb, in_=dn2[0].partition_broadcast(64))
                        muleng = nc.vector if hp == H // 2 - 1 else nc.gpsimd
                        muleng.tensor_mul(
                            catt[hp][half * 64:(half + 1) * 64,
                                     sc * 512:(sc + 1) * 512],
                            asb[0:64, :], rcb)

        # ---- P4: output projection ----
        with tc.tile_pool(name="osb2", bufs=3) as osb2, \
             tc.tile_pool(name="ps_o", bufs=4, space="PSUM") as ps_o:
            for m in range(NT):
                ps = ps_o.tile([128, E], f32, tag="po", name=f"po{m}")
                for k in range(NE):
                    for off, sz in ((0, 512), (512, E - 512)):
                        nc.tensor.matmul(
                            ps[:, off:off + sz],
                            lhsT=catt[k][:, m * 128:(m + 1) * 128],
                            rhs=wo[k][:, off:off + sz],
                            start=(k == 0), stop=(k == NE - 1),
                        )
                ot = osb2.tile([128, E], f32, tag="o", name=f"ot{m}")
                nc.vector.scalar_tensor_tensor(ot, ps, 1.0, bo_bc, MULT, ADD)
                nc.sync.dma_start(out=out_d[m * 128:(m + 1) * 128, :], in_=ot)

    _dedupe_ldweights(nc)
    if split_waits:
        _split_multiwaits(nc)
    return nc


def _prep_weights(Wq, bq, Wk, bk, Wv, bv, Wo, bo):
    bf16 = ml_dtypes.bfloat16
    scale = 1.0 / np.sqrt(np.float32(DH))

    wqt = (np.asarray(Wq, np.float32).reshape(H * DH, E) * scale).T.astype(bf16)
    wkt = np.asarray(Wk, np.float32).reshape(H * DH, E).T.astype(bf16)
    bqv = (np.asarray(bq, np.float32).reshape(E, 1) * scale).astype(np.float32)
    bkv = np.asarray(bk, np.float32).reshape(E, 1).astype(np.float32)

    wvt = np.zeros((E, VW), np.float32)
    Wv = np.asarray(Wv, np.float32)
    bv = np.asarray(bv, np.float32)
    for h in range(H):
        wvt[0:E, h * HW:h * HW + DH] = Wv[h].T
    wvt = wvt.astype(bf16)

    Wo = np.asarray(Wo, np.float32)
    bo = np.asarray(bo, np.float32)
    # fold the V bias through the output projection: softmax weights sum to
    # one, so attn(v + bv) = attn(v) + bv and out += Wo @ concat(bv)
    bv_cat = bv.reshape(E)
    bo_new = (bo + Wo @ bv_cat).reshape(1, E).astype(np.float32)
    wot = Wo.T.astype(bf16)
    return wqt, wkt, bqv, bkv, wvt, wot, bo_new


def _install_ntff_shim():
    """Provide antenv.axon_hooks (absent in this image) so trace=True can
    drive NRT profiling through libaxon_pjrt.so.  Dev-only; harmless no-op
    when anything is missing."""
    import sys, types
    try:
        import antenv.axon_hooks  # noqa
        return
    except ImportError:
        pass
    try:
        import antenv
        mod = types.ModuleType("antenv.axon_hooks")
        _state = {}
        mod.set_axon_ntff_profile_hook = lambda h: _state.update(h=h)
        mod.get_axon_ntff_profile_hook = lambda: _state.get("h")
        sys.modules["antenv.axon_hooks"] = mod
        antenv.axon_hooks = mod
        from trn_agent_boot.trn_boot import _ntff_profile_via_ctypes
        hook = _ntff_profile_via_ctypes("/opt/axon/libaxon_pjrt.so")
        if hook is not None:
            mod.set_axon_ntff_profile_hook(hook)
    except Exception as e:  # pragma: no cover
        print(f"ntff shim failed: {e}")


def kernel(x, Wq, bq, Wk, bk, Wv, bv, Wo, bo):
    from concourse.bass_utils import run_bass_kernel_spmd

    if "nc" not in _cache:
        _cache["nc"] = _build_bass()
    nc = _cache["nc"]

    wqt, wkt, bqv, bkv, wvt, wot, bo_new = _prep_weights(
        Wq, bq, Wk, bk, Wv, bv, Wo, bo)
    x = np.asarray(x, np.float32)
    in_maps = [
        {"x": np.ascontiguousarray(x[b]),
         "wqt": wqt, "wkt": wkt, "bq": bqv, "bk": bkv,
         "wvt": wvt, "wot": wot, "bo": bo_new}
        for b in range(B)
    ]
    trace = bool(int(os.environ.get("MHA_TRACE", "0")))
    if trace:
        _install_ntff_shim()
    res = run_bass_kernel_spmd(nc, in_maps, list(range(B)), trace=trace)
    _cache["last_results"] = res
    return np.stack([res.results[b]["out"] for b in range(B)]).astype(np.float32)
